# revision 3
# baseline (speedup 1.0000x reference)
"""Clifford self-attention TRN2 kernel.

B=4, S=4096, 8 blades. Full inputs in, full output out; internally sharded
over 8 NeuronCores: 2048 query rows per core (core c -> batch c//2, sequence
half c%2), with each core holding its batch's full sequence for K/V.

Math folding (host side, exact):
  clifford_linear(x, w, b) == x @ M + b  with M[j,k] = sum_i C[i,j,k] w[i]
  c0 = CAYLEY[...,0] is diagonal (+1 x4, -1 x4): logits = Q @ diag(c0)/2 @ K^T
     -> fold diag(c0)/2 into K'  (K' = x @ Mk' + bk')
  output proj folds into V:  (P@V)/denom @ Mo = P@(V@Mo)/denom
     -> V' = x @ (Mv@Mo) + bv@Mo, with a ones column appended so the PV
        matmul also accumulates the softmax denominator.

All attention matmuls run in fp32r (fp32 storage rounded to ~11 mantissa
bits, full PE rate; measured end-to-end error vs fp32 reference ~1e-3 of
output scale). Softmax skips max-subtraction: logits are ~N(0, 2), bounded
by ~12, so exp stays comfortably inside fp32 range; normalization divides
at the end by the ones-column accumulator.

Device program (per core), all scheduled by Tile:
  - DMA x into SBUF token-major, PE-transpose to blade-major xT [8, S]
  - QT/K'T/V1T = W.T @ xT via fp32r matmuls
  - V1 token-major [128, 9] tiles via PE transposes
  - flash loop over 4 query supertiles x 32 key tiles (groups of 3):
      logitsT [128k, 512q] (PSUM) -> Exp on ACT (fp32r out) -> PV matmul
      accumulating into [9, 512] PSUM (outputs 0..7 + denominator row 8)
  - PE-transpose [9,128] -> [128,9], DVE: reciprocal, scale, +bo, DMA out
"""

import sys

if "/opt/trn_rl_repo" not in sys.path:
    sys.path.insert(0, "/opt/trn_rl_repo")

import numpy as np

N_BLADES = 8
B, S = 4, 4096
NCORES = 8
QPC = B * S // NCORES  # queries per core = 2048
NQ = QPC // 128        # query token groups per partition = 16
NKV = S // 128         # kv token groups per partition = 32
NT = S // 128          # key tiles = 32
NST = QPC // 512       # query supertiles = 4
GROUP = 3              # key tiles per PSUM logits group (3 banks)


def _build_cayley():
    blades = [0, 1, 2, 4, 3, 5, 6, 7]
    idx = {b: i for i, b in enumerate(blades)}
    C = np.zeros((8, 8, 8), dtype=np.float32)
    for i, a in enumerate(blades):
        for j, b in enumerate(blades):
            aa = a >> 1
            cnt = 0
            while aa:
                cnt += bin(aa & b).count("1")
                aa >>= 1
            sign = -1.0 if (cnt & 1) else 1.0
            C[i, j, idx[a ^ b]] += sign
    return C


def _fold_weights(wq, bq, wk, bk, wv, bv, wo, bo):
    C = _build_cayley()
    c0d = np.diag(C[..., 0]).astype(np.float32)  # [+1 x4, -1 x4]
    Mq = np.einsum("ijk,i->jk", C, wq).astype(np.float32)
    Mk = np.einsum("ijk,i->jk", C, wk).astype(np.float32)
    Mv = np.einsum("ijk,i->jk", C, wv).astype(np.float32)
    Mo = np.einsum("ijk,i->jk", C, wo).astype(np.float32)
    Mkp = (Mk * (c0d[None, :] * 0.5)).astype(np.float32)
    bkp = (bk * c0d * 0.5).astype(np.float32)
    Mvp = (Mv @ Mo).astype(np.float32)
    bvp = (bv @ Mo).astype(np.float32)
    # V1 weights: [Mv' | 0] (8x9), bias [bv' ; 1] so row 8 of V1T is all ones
    Mv1 = np.concatenate([Mvp, np.zeros((8, 1), np.float32)], axis=1)
    bv1 = np.concatenate([bvp, np.ones(1, np.float32)])
    return {
        "mq": Mq,
        "mk": Mkp,
        "mv": Mv1,
        "bias_q": bq.reshape(8, 1).astype(np.float32),
        "bias_k": bkp.reshape(8, 1),
        "bias_v1": bv1.reshape(9, 1),
        "bo_b": np.broadcast_to(bo.astype(np.float32), (128, 8)).copy(),
    }


_CACHE = {}


def _compiled():
    if "nc" in _CACHE:
        return _CACHE["nc"]

    import concourse.bass as bass
    from concourse import bacc, masks, mybir, tile

    f32 = mybir.dt.float32
    f32r = mybir.dt.float32r
    Exp = mybir.ActivationFunctionType.Exp
    Ident = mybir.ActivationFunctionType.Identity

    nc = bacc.Bacc(
        "TRN2",
        target_bir_lowering=False,
        debug=False,
        enable_asserts=False,
        num_devices=NCORES,
    )

    xkv = nc.dram_tensor("xkv", [S, 8], f32, kind="ExternalInput").ap()
    xq = nc.dram_tensor("xq", [QPC, 8], f32, kind="ExternalInput").ap()
    dws = {
        nm: nc.dram_tensor(nm, shp, f32, kind="ExternalInput").ap()
        for nm, shp in [
            ("mq", [8, 8]),
            ("mk", [8, 8]),
            ("mv", [8, 9]),
            ("bias_q", [8, 1]),
            ("bias_k", [8, 1]),
            ("bias_v1", [9, 1]),
            ("bo_b", [128, 8]),
        ]
    }
    y = nc.dram_tensor("y", [QPC, 8], f32, kind="ExternalOutput").ap()

    with tile.TileContext(nc) as tc:
        with (
            tc.tile_pool(name="persist", bufs=1) as persist,
            tc.tile_pool(name="wpool", bufs=1) as wpool,
        ):
            ident = persist.tile([128, 128], f32)
            masks.make_identity(nc, ident[:])

            # stage weights (fp32), then round the matmul weights to fp32r
            w_sb = {}
            for nm, ap_ in dws.items():
                t = wpool.tile(list(ap_.shape), f32, name=f"st_{nm}")
                nc.sync.dma_start(t[:], ap_[:])
                w_sb[nm] = t
            w_r = {}
            for nm in ("mq", "mk", "mv"):
                t = wpool.tile(list(dws[nm].shape), f32r, name=f"wr_{nm}")
                nc.vector.tensor_copy(t[:], w_sb[nm][:])
                w_r[nm] = t

            xkvT = persist.tile([8, S], f32r)      # blade-major x (kv)
            xqT = persist.tile([8, QPC], f32r)     # blade-major x (q)
            kT = persist.tile([8, S], f32r)        # K' blade-major
            qT = persist.tile([8, QPC], f32r)      # Q blade-major
            v1T = persist.tile([9, S], f32)        # V' blade-major + ones row
            v1 = persist.tile([128, NT * 9], f32r) # V' token-major [128,9] tiles

            # ---------------- prep ----------------
            with (
                tc.tile_pool(name="prep_sb", bufs=1) as prep_sb,
                tc.tile_pool(name="prep_ps", bufs=2, space="PSUM") as prep_ps,
            ):
                # token-major loads: partition p holds tokens [p*n .. p*n+n)
                xkv_sb = prep_sb.tile([128, NKV * 8], f32)
                nc.sync.dma_start(
                    xkv_sb[:], xkv.rearrange("(p n) d -> p (n d)", p=128)
                )
                xq_sb = prep_sb.tile([128, NQ * 8], f32)
                nc.sync.dma_start(
                    xq_sb[:], xq.rearrange("(p n) d -> p (n d)", p=128)
                )

                # x transposes -> blade-major (4 per PSUM bank, then copy out)
                for dst, src, ngrp in ((xkvT, xkv_sb, NKV), (xqT, xq_sb, NQ)):
                    for b0 in range(0, ngrp, 4):
                        xt_ps = prep_ps.tile([8, 512], f32, tag="xt")
                        for k in range(4):
                            n = b0 + k
                            nc.tensor.transpose(
                                xt_ps[:, k * 128 : (k + 1) * 128],
                                src[:, n * 8 : (n + 1) * 8],
                                ident[:],
                            )
                        if (b0 // 4) % 2 == 0:
                            nc.vector.tensor_copy(
                                dst[:, b0 * 128 : (b0 + 4) * 128], xt_ps[:]
                            )
                        else:
                            nc.scalar.copy(
                                dst[:, b0 * 128 : (b0 + 4) * 128], xt_ps[:]
                            )

                # QKV projections (fp32r), + per-partition bias on copy-out
                projs = [
                    (qT, xqT, "mq", "bias_q", 8, QPC),
                    (kT, xkvT, "mk", "bias_k", 8, S),
                    (v1T, xkvT, "mv", "bias_v1", 9, S),
                ]
                for dst, srcT, wnm, bnm, mdim, width in projs:
                    for j in range(0, width, 512):
                        pps = prep_ps.tile([mdim, 512], f32, tag="pj")
                        nc.tensor.matmul(
                            pps[:], w_r[wnm][:], srcT[:, j : j + 512],
                            start=True, stop=True,
                        )
                        if (j // 512) % 2 == 0:
                            nc.vector.tensor_scalar_add(
                                dst[:, j : j + 512], pps[:], w_sb[bnm][:]
                            )
                        else:
                            nc.scalar.activation(
                                dst[:, j : j + 512], pps[:], Ident,
                                bias=w_sb[bnm][:],
                            )

                # V1 token-major tiles via PE transpose (fp32 -> fp32r copy)
                for b0 in range(0, NT, 4):
                    vt_ps = prep_ps.tile([128, 36], f32, tag="vt")
                    for k in range(4):
                        t = b0 + k
                        nc.tensor.transpose(
                            vt_ps[:, k * 9 : (k + 1) * 9],
                            v1T[:, t * 128 : (t + 1) * 128],
                            ident[:9, :9],
                        )
                    nc.vector.tensor_copy(
                        v1[:, b0 * 9 : (b0 + 4) * 9], vt_ps[:]
                    )

            # ---------------- main flash loop ----------------
            groups = [
                list(range(g, min(g + GROUP, NT))) for g in range(0, NT, GROUP)
            ]
            with (
                tc.tile_pool(name="lg_ps", bufs=2, space="PSUM") as lg_pool,
                tc.tile_pool(name="o_ps", bufs=2, space="PSUM") as o_pool,
                tc.tile_pool(name="pt_sb", bufs=3) as pt_pool,
                tc.tile_pool(name="fin_sb", bufs=4) as fin_pool,
            ):
                for s in range(NST):
                    q_rhs = qT[:, s * 512 : (s + 1) * 512]
                    o_ps = o_pool.tile([9, 512], f32, tag="o")
                    first_pv = True
                    for grp in groups:
                        gl = len(grp) * 512
                        lg = lg_pool.tile([128, GROUP * 512], f32, tag="lg")
                        for i, t in enumerate(grp):
                            nc.tensor.matmul(
                                lg[:, i * 512 : (i + 1) * 512],
                                kT[:, t * 128 : (t + 1) * 128],
                                q_rhs,
                                start=True, stop=True,
                            )
                        pt = pt_pool.tile([128, GROUP * 512], f32r, tag="pt")
                        nc.scalar.activation(pt[:, :gl], lg[:, :gl], Exp)
                        for i, t in enumerate(grp):
                            nc.tensor.matmul(
                                o_ps[:],
                                v1[:, t * 9 : (t + 1) * 9],
                                pt[:, i * 512 : (i + 1) * 512],
                                start=first_pv, stop=(t == NT - 1),
                            )
                            first_pv = False

                    # normalize + bias + store, one 128-query tile at a time
                    o_sb = fin_pool.tile([9, 512], f32, tag="osb")
                    nc.vector.tensor_copy(o_sb[:], o_ps[:])
                    for i in range(4):
                        ot = o_pool.tile([128, 16], f32, tag="o")
                        nc.tensor.transpose(
                            ot[:, 0:9],
                            o_sb[:, i * 128 : (i + 1) * 128],
                            ident[:9, :9],
                        )
                        rcp = fin_pool.tile([128, 1], f32, tag="rcp")
                        nc.vector.reciprocal(rcp[:], ot[:, 8:9])
                        yt = fin_pool.tile([128, 8], f32, tag="yt")
                        nc.vector.tensor_scalar_mul(yt[:], ot[:, 0:8], rcp[:])
                        nc.vector.tensor_add(yt[:], yt[:], w_sb["bo_b"][:])
                        nc.sync.dma_start(
                            y.rearrange("(p n) d -> p n d", p=128)[
                                :, s * 4 + i, :
                            ],
                            yt[:],
                        )

    nc.compile()
    _CACHE["nc"] = nc
    return nc


def kernel(x, wq, bq, wk, bk, wv, bv, wo, bo):
    from concourse import bass_utils

    x = np.ascontiguousarray(np.asarray(x, dtype=np.float32))
    assert x.shape == (B, S, N_BLADES), x.shape
    w = _fold_weights(
        *[
            np.asarray(a, dtype=np.float32)
            for a in (wq, bq, wk, bk, wv, bv, wo, bo)
        ]
    )

    nc = _compiled()
    in_maps = []
    for c in range(NCORES):
        b, h = c // 2, c % 2
        m = dict(w)
        m["xkv"] = x[b]
        m["xq"] = np.ascontiguousarray(x[b, h * QPC : (h + 1) * QPC])
        in_maps.append(m)

    res = bass_utils.run_bass_kernel_spmd(nc, in_maps, list(range(NCORES)))

    out = np.empty((B, S, N_BLADES), dtype=np.float32)
    for c in range(NCORES):
        b, h = c // 2, c % 2
        out[b, h * QPC : (h + 1) * QPC] = res.results[c]["y"]
    return out


# revision 6
# speedup vs baseline: 1.3412x; 1.3412x over previous
"""Clifford self-attention TRN2 kernel.

B=4, S=4096, 8 blades. Full inputs in, full output out; internally sharded
over 8 NeuronCores: 2048 query rows per core (core c -> batch c//2, sequence
half c%2), with each core holding its batch's full sequence for K/V.

Math folding (host side, exact):
  clifford_linear(x, w, b) == x @ M + b  with M[j,k] = sum_i C[i,j,k] w[i]
  c0 = CAYLEY[...,0] is diagonal (+1 x4, -1 x4): logits = Q @ diag(c0)/2 @ K^T
     -> fold diag(c0)/2 into K'  (K' = x @ Mk' + bk')
  output proj folds into V:  (P@V)/denom @ Mo = P@(V@Mo)/denom
     -> V' = x @ (Mv@Mo) + bv@Mo, with a ones column appended so the PV
        matmul also accumulates the softmax denominator.

All attention matmuls run in fp32r (fp32 storage rounded to ~11 mantissa
bits, full PE rate; measured end-to-end error vs fp32 reference ~4e-4 of
output scale). The 8-wide contraction is zero-padded to 128 partitions:
matmul cost depends only on the moving free dim, and K=8 matmuls keep the
PE's HAM activity monitor cold (half clock) while K=128 runs at 2.4 GHz.
Softmax skips max-subtraction: logits are ~N(0, 2), bounded by ~12, so exp
stays comfortably inside fp32 range; normalization divides at the end by
the ones-column accumulator.

Device program (per core), all scheduled by Tile:
  - DMA x into SBUF token-major, PE-transpose to blade-major xT [128, S]
    (rows 8+ zeroed by an upfront memset)
  - QT/K'T/V1T = W.T @ xT via fp32r matmuls with [128,128] zero-padded W
  - V1 token-major [128, 9] tiles via K=128 PE transposes
  - flash loop over 4 query supertiles x 32 key tiles (groups of 3):
      logitsT [128k, 512q] (PSUM) -> Exp on ACT (fp32r out) -> PV matmul
      accumulating into [9, 512] PSUM (outputs 0..7 + denominator row 8)
  - PE-transpose [9,128] -> [128,9], DVE: reciprocal, scale, +bo, DMA out
"""

import sys

if "/opt/trn_rl_repo" not in sys.path:
    sys.path.insert(0, "/opt/trn_rl_repo")

import numpy as np

N_BLADES = 8
B, S = 4, 4096
NCORES = 8
QPC = B * S // NCORES  # queries per core = 2048
NQ = QPC // 128        # query token groups per partition = 16
NKV = S // 128         # kv token groups per partition = 32
NT = S // 128          # key tiles = 32
NST = QPC // 512       # query supertiles = 4
GROUP = 3              # key tiles per PSUM logits group (3 banks)


def _build_cayley():
    blades = [0, 1, 2, 4, 3, 5, 6, 7]
    idx = {b: i for i, b in enumerate(blades)}
    C = np.zeros((8, 8, 8), dtype=np.float32)
    for i, a in enumerate(blades):
        for j, b in enumerate(blades):
            aa = a >> 1
            cnt = 0
            while aa:
                cnt += bin(aa & b).count("1")
                aa >>= 1
            sign = -1.0 if (cnt & 1) else 1.0
            C[i, j, idx[a ^ b]] += sign
    return C


def _fold_weights(wq, bq, wk, bk, wv, bv, wo, bo):
    C = _build_cayley()
    c0d = np.diag(C[..., 0]).astype(np.float32)  # [+1 x4, -1 x4]
    Mq = np.einsum("ijk,i->jk", C, wq).astype(np.float32)
    Mk = np.einsum("ijk,i->jk", C, wk).astype(np.float32)
    Mv = np.einsum("ijk,i->jk", C, wv).astype(np.float32)
    Mo = np.einsum("ijk,i->jk", C, wo).astype(np.float32)
    Mkp = (Mk * (c0d[None, :] * 0.5)).astype(np.float32)
    bkp = (bk * c0d * 0.5).astype(np.float32)
    Mvp = (Mv @ Mo).astype(np.float32)
    bvp = (bv @ Mo).astype(np.float32)

    # zero-pad weights to [128, 128]: contraction rows 8+ and output cols
    # beyond the real width are 0, so the projection matmuls write exact
    # zeros into the padded rows of QT/K'T/V1T.
    def pad(m):
        out = np.zeros((128, 128), np.float32)
        out[: m.shape[0], : m.shape[1]] = m
        return out

    def padb(v):
        out = np.zeros((128, 1), np.float32)
        out[: v.size, 0] = v
        return out

    bv1 = np.concatenate([bvp, np.ones(1, np.float32)])  # row 8 -> ones row
    Mv1 = np.concatenate([Mvp, np.zeros((8, 1), np.float32)], axis=1)
    return {
        "mq": pad(Mq),
        "mk": pad(Mkp),
        "mv": pad(Mv1),
        "bias_q": padb(bq.astype(np.float32)),
        "bias_k": padb(bkp),
        "bias_v1": padb(bv1),
        "bo_b": np.broadcast_to(bo.astype(np.float32), (128, 8)).copy(),
    }


_CACHE = {}


def _compiled():
    if "nc" in _CACHE:
        return _CACHE["nc"]

    import concourse.bass as bass
    from concourse import bacc, masks, mybir, tile

    f32 = mybir.dt.float32
    f32r = mybir.dt.float32r
    Exp = mybir.ActivationFunctionType.Exp

    nc = bacc.Bacc(
        "TRN2",
        target_bir_lowering=False,
        debug=False,
        enable_asserts=False,
        num_devices=NCORES,
    )

    xkv = nc.dram_tensor("xkv", [S, 8], f32, kind="ExternalInput").ap()
    xq = nc.dram_tensor("xq", [QPC, 8], f32, kind="ExternalInput").ap()
    dws = {
        nm: nc.dram_tensor(nm, shp, f32, kind="ExternalInput").ap()
        for nm, shp in [
            ("mq", [128, 128]),
            ("mk", [128, 128]),
            ("mv", [128, 128]),
            ("bias_q", [128, 1]),
            ("bias_k", [128, 1]),
            ("bias_v1", [128, 1]),
            ("bo_b", [128, 8]),
        ]
    }
    y = nc.dram_tensor("y", [QPC, 8], f32, kind="ExternalOutput").ap()

    with tile.TileContext(nc) as tc:
        with (
            tc.tile_pool(name="persist", bufs=1) as persist,
            tc.tile_pool(name="wpool", bufs=1) as wpool,
        ):
            ident = persist.tile([128, 128], f32)
            masks.make_identity(nc, ident[:])

            # stage weights (fp32), then round the matmul weights to fp32r
            w_sb = {}
            for nm, ap_ in dws.items():
                t = wpool.tile(list(ap_.shape), f32, name=f"st_{nm}")
                nc.sync.dma_start(t[:], ap_[:])
                w_sb[nm] = t
            w_r = {}
            for nm in ("mq", "mk", "mv"):
                t = wpool.tile([128, 128], f32r, name=f"wr_{nm}")
                nc.vector.tensor_copy(t[:], w_sb[nm][:])
                w_r[nm] = t

            xkvT = persist.tile([128, S], f32r)    # blade-major x (kv)
            xqT = persist.tile([128, QPC], f32r)   # blade-major x (q)
            kT = persist.tile([128, S], f32r)      # K' blade-major
            qT = persist.tile([128, QPC], f32r)    # Q blade-major
            v1T = persist.tile([128, S], f32)      # V' blade-major + ones row
            v1 = persist.tile([128, NT * 9], f32r) # V' token-major [128,9] tiles

            # ---------------- prep ----------------
            with (
                tc.tile_pool(name="prep_sb", bufs=1) as prep_sb,
                tc.tile_pool(name="prep_ps", bufs=2, space="PSUM") as prep_ps,
            ):
                # rows 8+ of xT are contraction padding: the weights there
                # are zero, but junk SBUF could hold NaN (0*NaN=NaN), so
                # clear them once. fp32r can't be memset directly (ISA), so
                # round-copy from a zeroed fp32 tile. Transposes/copies then
                # fill rows 0..7.
                zeros_sb = prep_sb.tile([128, 512], f32)
                nc.gpsimd.memset(zeros_sb[:], 0.0)
                for j in range(0, S, 512):
                    nc.vector.tensor_copy(xkvT[:, j : j + 512], zeros_sb[:])
                for j in range(0, QPC, 512):
                    nc.scalar.copy(xqT[:, j : j + 512], zeros_sb[:])

                # token-major loads: partition p holds tokens [p*n .. p*n+n)
                xkv_sb = prep_sb.tile([128, NKV * 8], f32)
                nc.sync.dma_start(
                    xkv_sb[:], xkv.rearrange("(p n) d -> p (n d)", p=128)
                )
                xq_sb = prep_sb.tile([128, NQ * 8], f32)
                nc.sync.dma_start(
                    xq_sb[:], xq.rearrange("(p n) d -> p (n d)", p=128)
                )

                # x transposes -> blade-major (4 per PSUM bank, then copy out)
                for dst, src, ngrp in ((xkvT, xkv_sb, NKV), (xqT, xq_sb, NQ)):
                    for b0 in range(0, ngrp, 4):
                        xt_ps = prep_ps.tile([8, 512], f32, tag="xt")
                        for k in range(4):
                            n = b0 + k
                            nc.tensor.transpose(
                                xt_ps[:, k * 128 : (k + 1) * 128],
                                src[:, n * 8 : (n + 1) * 8],
                                ident[:],
                            )
                        if (b0 // 4) % 2 == 0:
                            nc.vector.tensor_copy(
                                dst[0:8, b0 * 128 : (b0 + 4) * 128], xt_ps[:]
                            )
                        else:
                            nc.scalar.copy(
                                dst[0:8, b0 * 128 : (b0 + 4) * 128], xt_ps[:]
                            )

                # QKV projections (fp32r), + per-partition bias on copy-out.
                # Padded weights give M=128 outputs whose rows 8+ are exact
                # zeros, so kT/qT need no separate zeroing.
                projs = [
                    (qT, xqT, "mq", "bias_q", QPC),
                    (kT, xkvT, "mk", "bias_k", S),
                    (v1T, xkvT, "mv", "bias_v1", S),
                ]
                for dst, srcT, wnm, bnm, width in projs:
                    for j in range(0, width, 512):
                        pps = prep_ps.tile([128, 512], f32, tag="pj")
                        nc.tensor.matmul(
                            pps[:], w_r[wnm][:], srcT[:, j : j + 512],
                            start=True, stop=True,
                        )
                        nc.vector.tensor_scalar_add(
                            dst[:, j : j + 512], pps[:], w_sb[bnm][:]
                        )

                # V1 token-major tiles via K=128 PE transpose (rows 9+ of
                # v1T are zero, transposing junk-free)
                for b0 in range(0, NT, 2):
                    vt_ps = prep_ps.tile([128, 256], f32, tag="vt")
                    for k in range(2):
                        t = b0 + k
                        nc.tensor.transpose(
                            vt_ps[:, k * 128 : (k + 1) * 128],
                            v1T[:, t * 128 : (t + 1) * 128],
                            ident[:],
                        )
                    for k in range(2):
                        t = b0 + k
                        nc.vector.tensor_copy(
                            v1[:, t * 9 : (t + 1) * 9],
                            vt_ps[:, k * 128 : k * 128 + 9],
                        )

            # ---------------- main flash loop ----------------
            groups = [
                list(range(g, min(g + GROUP, NT))) for g in range(0, NT, GROUP)
            ]
            with (
                tc.tile_pool(name="lg_ps", bufs=2, space="PSUM") as lg_pool,
                tc.tile_pool(name="o_ps", bufs=2, space="PSUM") as o_pool,
                tc.tile_pool(name="pt_sb", bufs=3) as pt_pool,
                tc.tile_pool(name="fin_sb", bufs=4) as fin_pool,
            ):
                for s in range(NST):
                    q_rhs = qT[:, s * 512 : (s + 1) * 512]
                    o_ps = o_pool.tile([9, 512], f32, tag="o")
                    first_pv = True
                    for grp in groups:
                        gl = len(grp) * 512
                        lg = lg_pool.tile([128, GROUP * 512], f32, tag="lg")
                        for i, t in enumerate(grp):
                            nc.tensor.matmul(
                                lg[:, i * 512 : (i + 1) * 512],
                                kT[:, t * 128 : (t + 1) * 128],
                                q_rhs,
                                start=True, stop=True,
                            )
                        pt = pt_pool.tile([128, GROUP * 512], f32r, tag="pt")
                        nc.scalar.activation(pt[:, :gl], lg[:, :gl], Exp)
                        for i, t in enumerate(grp):
                            nc.tensor.matmul(
                                o_ps[:],
                                v1[:, t * 9 : (t + 1) * 9],
                                pt[:, i * 512 : (i + 1) * 512],
                                start=first_pv, stop=(t == NT - 1),
                            )
                            first_pv = False

                    # normalize + bias + store, one 128-query tile at a time
                    o_sb = fin_pool.tile([9, 512], f32, tag="osb")
                    nc.vector.tensor_copy(o_sb[:], o_ps[:])
                    for i in range(4):
                        ot = o_pool.tile([128, 16], f32, tag="o")
                        nc.tensor.transpose(
                            ot[:, 0:9],
                            o_sb[:, i * 128 : (i + 1) * 128],
                            ident[:9, :9],
                        )
                        rcp = fin_pool.tile([128, 1], f32, tag="rcp")
                        nc.vector.reciprocal(rcp[:], ot[:, 8:9])
                        yt = fin_pool.tile([128, 8], f32, tag="yt")
                        nc.vector.tensor_scalar_mul(yt[:], ot[:, 0:8], rcp[:])
                        nc.vector.tensor_add(yt[:], yt[:], w_sb["bo_b"][:])
                        nc.sync.dma_start(
                            y.rearrange("(p n) d -> p n d", p=128)[
                                :, s * 4 + i, :
                            ],
                            yt[:],
                        )

    nc.compile()
    _CACHE["nc"] = nc
    return nc


def kernel(x, wq, bq, wk, bk, wv, bv, wo, bo):
    from concourse import bass_utils

    x = np.ascontiguousarray(np.asarray(x, dtype=np.float32))
    assert x.shape == (B, S, N_BLADES), x.shape
    w = _fold_weights(
        *[
            np.asarray(a, dtype=np.float32)
            for a in (wq, bq, wk, bk, wv, bv, wo, bo)
        ]
    )

    nc = _compiled()
    in_maps = []
    for c in range(NCORES):
        b, h = c // 2, c % 2
        m = dict(w)
        m["xkv"] = x[b]
        m["xq"] = np.ascontiguousarray(x[b, h * QPC : (h + 1) * QPC])
        in_maps.append(m)

    res = bass_utils.run_bass_kernel_spmd(nc, in_maps, list(range(NCORES)))

    out = np.empty((B, S, N_BLADES), dtype=np.float32)
    for c in range(NCORES):
        b, h = c // 2, c % 2
        out[b, h * QPC : (h + 1) * QPC] = res.results[c]["y"]
    return out


# revision 9
# speedup vs baseline: 1.4081x; 1.0498x over previous
"""Clifford self-attention TRN2 kernel.

B=4, S=4096, 8 blades. Full inputs in, full output out; internally sharded
over 8 NeuronCores: 2048 query rows per core (core c -> batch c//2, sequence
half c%2), with each core holding its batch's full sequence for K/V.

Math folding (host side, exact):
  clifford_linear(x, w, b) == x @ M + b  with M[j,k] = sum_i C[i,j,k] w[i]
  c0 = CAYLEY[...,0] is diagonal (+1 x4, -1 x4): logits = Q @ diag(c0)/2 @ K^T
     -> fold diag(c0)/2 into K'  (K' = x @ Mk' + bk')
  output proj folds into V:  (P@V)/denom @ Mo = P@(V@Mo)/denom
     -> V' = x @ (Mv@Mo) + bv@Mo, with a ones column appended so the PV
        matmul also accumulates the softmax denominator.

All attention matmuls run in fp32r (fp32 storage rounded to ~11 mantissa
bits, full PE rate; measured end-to-end error vs fp32 reference ~4e-4 of
output scale). The 8-wide contraction is zero-padded to 128 partitions:
matmul cost depends only on the moving free dim, and K=8 matmuls keep the
PE's HAM activity monitor cold (half clock) while K=128 runs at 2.4 GHz.
Softmax skips max-subtraction: logits are ~N(0, 2), bounded by ~12, so exp
stays comfortably inside fp32 range; normalization divides at the end by
the ones-column accumulator.

Device program (per core), all scheduled by Tile:
  - DMA x into SBUF token-major, PE-transpose to blade-major xT [128, S]
    (rows 8+ zeroed by an upfront memset)
  - QT/K'T/V1T = W.T @ xT via fp32r matmuls with [128,128] zero-padded W
  - V1 token-major [128, 9] tiles via K=128 PE transposes
  - flash loop over 4 query supertiles x 32 key tiles (groups of 3):
      logitsT [128k, 512q] (PSUM) -> Exp on ACT (fp32r out) -> PV matmul
      accumulating into [9, 512] PSUM (outputs 0..7 + denominator row 8)
  - PE-transpose [9,128] -> [128,9], DVE: reciprocal, scale, +bo, DMA out
"""

import sys

if "/opt/trn_rl_repo" not in sys.path:
    sys.path.insert(0, "/opt/trn_rl_repo")

import numpy as np

N_BLADES = 8
B, S = 4, 4096
NCORES = 8
QPC = B * S // NCORES  # queries per core = 2048
NQ = QPC // 128        # query token groups per partition = 16
NKV = S // 128         # kv token groups per partition = 32
NT = S // 128          # key tiles = 32
NST = QPC // 512       # query supertiles = 4
GROUP = 3              # key tiles per PSUM logits group (3 banks)


def _build_cayley():
    blades = [0, 1, 2, 4, 3, 5, 6, 7]
    idx = {b: i for i, b in enumerate(blades)}
    C = np.zeros((8, 8, 8), dtype=np.float32)
    for i, a in enumerate(blades):
        for j, b in enumerate(blades):
            aa = a >> 1
            cnt = 0
            while aa:
                cnt += bin(aa & b).count("1")
                aa >>= 1
            sign = -1.0 if (cnt & 1) else 1.0
            C[i, j, idx[a ^ b]] += sign
    return C


def _fold_weights(wq, bq, wk, bk, wv, bv, wo, bo):
    C = _build_cayley()
    c0d = np.diag(C[..., 0]).astype(np.float32)  # [+1 x4, -1 x4]
    Mq = np.einsum("ijk,i->jk", C, wq).astype(np.float32)
    Mk = np.einsum("ijk,i->jk", C, wk).astype(np.float32)
    Mv = np.einsum("ijk,i->jk", C, wv).astype(np.float32)
    Mo = np.einsum("ijk,i->jk", C, wo).astype(np.float32)
    Mkp = (Mk * (c0d[None, :] * 0.5)).astype(np.float32)
    bkp = (bk * c0d * 0.5).astype(np.float32)
    Mvp = (Mv @ Mo).astype(np.float32)
    bvp = (bv @ Mo).astype(np.float32)

    # zero-pad weights to [128, 128]: contraction rows 8+ and output cols
    # beyond the real width are 0, so the projection matmuls write exact
    # zeros into the padded rows of QT/K'T/V1T.
    def pad(m):
        out = np.zeros((128, 128), np.float32)
        out[: m.shape[0], : m.shape[1]] = m
        return out

    def padb(v):
        out = np.zeros((128, 1), np.float32)
        out[: v.size, 0] = v
        return out

    bv1 = np.concatenate([bvp, np.ones(1, np.float32)])  # row 8 -> ones row
    Mv1 = np.concatenate([Mvp, np.zeros((8, 1), np.float32)], axis=1)
    return {
        "mq": pad(Mq),
        "mk": pad(Mkp),
        "mv": pad(Mv1),
        "bias_q": padb(bq.astype(np.float32)),
        "bias_k": padb(bkp),
        "bias_v1": padb(bv1),
        "bo_b": np.broadcast_to(bo.astype(np.float32), (128, 8)).copy(),
    }


_CACHE = {}


def _compiled():
    if "nc" in _CACHE:
        return _CACHE["nc"]

    import concourse.bass as bass
    from concourse import bacc, masks, mybir, tile

    f32 = mybir.dt.float32
    f32r = mybir.dt.float32r
    Exp = mybir.ActivationFunctionType.Exp

    nc = bacc.Bacc(
        "TRN2",
        target_bir_lowering=False,
        debug=False,
        enable_asserts=False,
        num_devices=NCORES,
    )

    xkv = nc.dram_tensor("xkv", [S, 8], f32, kind="ExternalInput").ap()
    xq = nc.dram_tensor("xq", [QPC, 8], f32, kind="ExternalInput").ap()
    dws = {
        nm: nc.dram_tensor(nm, shp, f32, kind="ExternalInput").ap()
        for nm, shp in [
            ("mq", [128, 128]),
            ("mk", [128, 128]),
            ("mv", [128, 128]),
            ("bias_q", [128, 1]),
            ("bias_k", [128, 1]),
            ("bias_v1", [128, 1]),
            ("bo_b", [128, 8]),
        ]
    }
    y = nc.dram_tensor("y", [QPC, 8], f32, kind="ExternalOutput").ap()

    with tile.TileContext(nc) as tc:
        with (
            tc.tile_pool(name="persist", bufs=1) as persist,
            tc.tile_pool(name="wpool", bufs=1) as wpool,
        ):
            ident = persist.tile([128, 128], f32)
            masks.make_identity(nc, ident[:])

            # x loads first (largest DMAs; transposes gate on them)
            xkv_sb = wpool.tile([128, NKV * 8], f32)
            nc.sync.dma_start(
                xkv_sb[:], xkv.rearrange("(p n) d -> p (n d)", p=128)
            )
            xq_sb = wpool.tile([128, NQ * 8], f32)
            nc.sync.dma_start(
                xq_sb[:], xq.rearrange("(p n) d -> p (n d)", p=128)
            )

            # stage weights (fp32), then round the matmul weights to fp32r
            w_sb = {}
            for nm, ap_ in dws.items():
                t = wpool.tile(list(ap_.shape), f32, name=f"st_{nm}")
                nc.sync.dma_start(t[:], ap_[:])
                w_sb[nm] = t
            w_r = {}
            for nm in ("mq", "mk", "mv"):
                t = wpool.tile([128, 128], f32r, name=f"wr_{nm}")
                nc.vector.tensor_copy(t[:], w_sb[nm][:])
                w_r[nm] = t

            xkvT = persist.tile([128, S], f32r)    # blade-major x (kv)
            xqT = persist.tile([128, QPC], f32r)   # blade-major x (q)
            kT = persist.tile([128, S], f32r)      # K' blade-major
            qT = persist.tile([128, QPC], f32r)    # Q blade-major
            v1T = persist.tile([128, S], f32)      # V' blade-major + ones row
            v1 = persist.tile([128, NT * 9], f32r) # V' token-major [128,9] tiles

            # ---------------- prep ----------------
            with (
                tc.tile_pool(name="prep_sb", bufs=1) as prep_sb,
                tc.tile_pool(name="prep_ps", bufs=2, space="PSUM") as prep_ps,
            ):
                # rows 8+ of xT are contraction padding: the weights there
                # are zero, but junk SBUF could hold NaN (0*NaN=NaN), so
                # clear them once. fp32r can't be memset directly (ISA), so
                # round-copy from a zeroed fp32 tile. Transposes/copies then
                # fill rows 0..7.
                zeros_sb = prep_sb.tile([128, 512], f32)
                nc.gpsimd.memset(zeros_sb[:], 0.0)
                for j in range(0, S, 512):
                    nc.vector.tensor_copy(xkvT[:, j : j + 512], zeros_sb[:])
                for j in range(0, QPC, 512):
                    nc.scalar.copy(xqT[:, j : j + 512], zeros_sb[:])

                # x transposes -> blade-major (4 per PSUM bank, then copy out)
                for dst, src, ngrp in ((xkvT, xkv_sb, NKV), (xqT, xq_sb, NQ)):
                    for b0 in range(0, ngrp, 4):
                        xt_ps = prep_ps.tile([8, 512], f32, tag="xt")
                        for k in range(4):
                            n = b0 + k
                            nc.tensor.transpose(
                                xt_ps[:, k * 128 : (k + 1) * 128],
                                src[:, n * 8 : (n + 1) * 8],
                                ident[:],
                            )
                        if (b0 // 4) % 2 == 0:
                            nc.vector.tensor_copy(
                                dst[0:8, b0 * 128 : (b0 + 4) * 128], xt_ps[:]
                            )
                        else:
                            nc.scalar.copy(
                                dst[0:8, b0 * 128 : (b0 + 4) * 128], xt_ps[:]
                            )

                # QKV projections (fp32r), + per-partition bias on copy-out.
                # Padded weights give M=128 outputs whose rows 8+ are exact
                # zeros, so kT/qT need no separate zeroing.
                projs = [
                    (qT, xqT, "mq", "bias_q", QPC),
                    (kT, xkvT, "mk", "bias_k", S),
                    (v1T, xkvT, "mv", "bias_v1", S),
                ]
                for dst, srcT, wnm, bnm, width in projs:
                    for j in range(0, width, 512):
                        pps = prep_ps.tile([128, 512], f32, tag="pj")
                        nc.tensor.matmul(
                            pps[:], w_r[wnm][:], srcT[:, j : j + 512],
                            start=True, stop=True,
                        )
                        nc.vector.tensor_scalar_add(
                            dst[:, j : j + 512], pps[:], w_sb[bnm][:]
                        )

                # V1 token-major tiles via K=128 PE transpose (rows 9+ of
                # v1T are zero, transposing junk-free)
                for b0 in range(0, NT, 2):
                    vt_ps = prep_ps.tile([128, 256], f32, tag="vt")
                    for k in range(2):
                        t = b0 + k
                        nc.tensor.transpose(
                            vt_ps[:, k * 128 : (k + 1) * 128],
                            v1T[:, t * 128 : (t + 1) * 128],
                            ident[:],
                        )
                    for k in range(2):
                        t = b0 + k
                        nc.vector.tensor_copy(
                            v1[:, t * 9 : (t + 1) * 9],
                            vt_ps[:, k * 128 : k * 128 + 9],
                        )

            # ---------------- main flash loop ----------------
            # software-pipelined emission: PV for group g is emitted one
            # group late, so the PE's in-order stream never waits on the
            # exp it just triggered — logits(g+1) runs while ACT does
            # exp(g). Final normalization is deferred past the last PV so
            # the o-pool PSUM slots never stall the steady-state stream.
            groups = [
                (s, list(range(g, min(g + GROUP, NT))))
                for s in range(NST)
                for g in range(0, NT, GROUP)
            ]
            o_sb = persist.tile([9, NST * 512], f32)
            with (
                tc.tile_pool(name="lg_ps", bufs=2, space="PSUM") as lg_pool,
                tc.tile_pool(name="o_ps", bufs=2, space="PSUM") as o_pool,
                tc.tile_pool(name="pt_sb", bufs=3) as pt_pool,
                tc.tile_pool(name="fin_sb", bufs=4) as fin_pool,
            ):
                o_ps = {}
                pend = []  # (s, grp, pt) awaiting their PV matmuls

                def emit_pv():
                    s, grp, pt = pend.pop(0)
                    for i, t in enumerate(grp):
                        nc.tensor.matmul(
                            o_ps[s][:],
                            v1[:, t * 9 : (t + 1) * 9],
                            pt[:, i * 512 : (i + 1) * 512],
                            start=(t == 0), stop=(t == NT - 1),
                        )
                    if grp[-1] == NT - 1:
                        nc.vector.tensor_copy(
                            o_sb[:, s * 512 : (s + 1) * 512], o_ps[s][:]
                        )

                for s, grp in groups:
                    if grp[0] == 0:
                        o_ps[s] = o_pool.tile(
                            [9, 512], f32, tag="o", name=f"o_ps{s}"
                        )
                    gl = len(grp) * 512
                    lg = lg_pool.tile([128, GROUP * 512], f32, tag="lg")
                    q_rhs = qT[:, s * 512 : (s + 1) * 512]
                    for i, t in enumerate(grp):
                        nc.tensor.matmul(
                            lg[:, i * 512 : (i + 1) * 512],
                            kT[:, t * 128 : (t + 1) * 128],
                            q_rhs,
                            start=True, stop=True,
                        )
                    pt = pt_pool.tile([128, GROUP * 512], f32r, tag="pt")
                    nc.scalar.activation(pt[:, :gl], lg[:, :gl], Exp)
                    pend.append((s, grp, pt))
                    if len(pend) > 1:
                        emit_pv()
                while pend:
                    emit_pv()

                # normalize + bias + store, one 128-query tile at a time
                for si in range(NST * 4):
                    ot = o_pool.tile([128, 16], f32, tag="o")
                    nc.tensor.transpose(
                        ot[:, 0:9],
                        o_sb[:, si * 128 : (si + 1) * 128],
                        ident[:9, :9],
                    )
                    rcp = fin_pool.tile([128, 1], f32, tag="rcp")
                    nc.vector.reciprocal(rcp[:], ot[:, 8:9])
                    yt = fin_pool.tile([128, 8], f32, tag="yt")
                    nc.vector.tensor_scalar_mul(yt[:], ot[:, 0:8], rcp[:])
                    nc.vector.tensor_add(yt[:], yt[:], w_sb["bo_b"][:])
                    nc.sync.dma_start(
                        y.rearrange("(p n) d -> p n d", p=128)[:, si, :],
                        yt[:],
                    )

    nc.compile()
    _CACHE["nc"] = nc
    return nc


def kernel(x, wq, bq, wk, bk, wv, bv, wo, bo):
    from concourse import bass_utils

    x = np.ascontiguousarray(np.asarray(x, dtype=np.float32))
    assert x.shape == (B, S, N_BLADES), x.shape
    w = _fold_weights(
        *[
            np.asarray(a, dtype=np.float32)
            for a in (wq, bq, wk, bk, wv, bv, wo, bo)
        ]
    )

    nc = _compiled()
    in_maps = []
    for c in range(NCORES):
        b, h = c // 2, c % 2
        m = dict(w)
        m["xkv"] = x[b]
        m["xq"] = np.ascontiguousarray(x[b, h * QPC : (h + 1) * QPC])
        in_maps.append(m)

    res = bass_utils.run_bass_kernel_spmd(nc, in_maps, list(range(NCORES)))

    out = np.empty((B, S, N_BLADES), dtype=np.float32)
    for c in range(NCORES):
        b, h = c // 2, c % 2
        out[b, h * QPC : (h + 1) * QPC] = res.results[c]["y"]
    return out
